# revision 27
# baseline (speedup 1.0000x reference)
"""CARAFE-Downsample Trainium2 kernel (8 NeuronCores, data-parallel over batch).

Problem (hardcoded shapes): x [8, 256, 128, 128] f32; 1x1-conv compressor ->
cx [8, 64, 128, 128]; 3x3 stride-2 conv encoder -> mask [8, 25, 64, 64];
softmax(mask * exp(p)) over the 25 taps; 5x5 stride-2 weighted reassembly of x
-> out [8, 256, 64, 64].

v2 strategy (vs the slab baseline):
 - one sample per core (B == n_cores == 8).
 - x ships to the device TWICE in bf16 (16 MiB total, vs 29 MiB before):
   once channel-major (xc, for the compressor matmul) and once as 64
   "row-pair" tiles XT[rp] = [128p, 2par, 256c] where partition
   p = 64*half + w'' holds x[c, rp + 64*half, 2*w'' + par].  Every 5x5
   reassembly tap for pixel block k (pixels (h'=k, w') and (h'=k+32, w') on
   the 128 partitions) is then a +-1-partition-shifted slice of one tile:
   tap (i, j) reads XT[2k+i-2][par=j&1] at partition shift
   {0:-1,1:-1,2:0,3:0,4:+1}[j].
 - out-of-image taps are handled by zeroing their softmax weights (a 0/1
   bmask folded into the softmax normalize STT) so shifted reads can clamp
   to legal partition ranges; border blocks 0/31 route their out-of-tile
   taps to DVE/GPSIMD sub-range ops.
 - phase D (reassembly) is split across all four engines: most taps run as
   PE diag-matmuls (lhsT = diag(w_t), built by one tensor_scalar on
   ACT/GPSIMD/DVE from host-sent identity tiles - a shifted identity isup
   bakes the shifted diagonal for the delta=-1 taps), the rest as
   scalar_tensor_tensor fused-MAC chains on DVE and GPSIMD.  Per block the
   three partial accumulators (psum, accD, accG) merge on DVE and DMA out
   in bf16.
 - mask path (compressor + encoder) runs in bf16 on PE as before; psum
   evacuations are split ACT/DVE/GPSIMD to balance engine load.
"""

import numpy as np
import ml_dtypes

import concourse.bass as bass
import concourse.bacc as bacc
import concourse.tile as tile
from concourse import mybir
from concourse.bass_utils import run_bass_kernel_spmd

# -- problem constants (hardcoded per spec) ---------------------------------
B, C, H, W = 8, 256, 128, 128
CC = 64           # compressed channels
KK = 5            # CARAFE window
HP = WP = 64      # output spatial
NB = 32           # pixel blocks per sample
NCORES = 8

X_DTYPE = "bf16"
MASK_DTYPE = "bf16"

_DTM = {"f32": mybir.dt.float32, "bf16": mybir.dt.bfloat16}
_NPM = {"f32": np.float32, "bf16": ml_dtypes.bfloat16}
DTX, DTK = _DTM[X_DTYPE], _DTM[MASK_DTYPE]
NPX, NPK = _NPM[X_DTYPE], _NPM[MASK_DTYPE]
F32 = mybir.dt.float32

# column shift per j (partition shift within a parity plane)
DELTA = {0: -1, 1: -1, 2: 0, 3: 0, 4: 1}

# -- phase D tap -> engine assignment (tunable) -----------------------------
# 25 taps: "P" = PE diag-matmul, "V" = DVE STT chain, "G" = GPSIMD STT chain.
# Engine ops need partition-aligned operands, so DVE/GPSIMD only take
# delta==0 taps (j in {2, 3}); PE takes all shifted taps (j in {0, 1, 4})
# via sub/super-diagonal lhsT built from shifted identities and
# DMA-shifted weight columns.
TAP_ENGINE = {}
for _t in range(25):
    TAP_ENGINE[_t] = "P"
for _t in (3, 7, 8, 13, 17, 18, 22, 23):   # delta-0 taps for DVE
    TAP_ENGINE[_t] = "V"
# (GPSIMD cannot run scalar-AP ops; it only builds diag tiles via
# tensor_tensor with a stride-0 broadcast weight AP)

# builder engine for each PE tap (in issue order): A=ACT, G=GPSIMD, D=DVE
BUILDERS = ["G", "A", "G", "A", "D", "G", "A", "G", "A", "D", "G", "A", "G", "A", "D", "G", "A"]

# psum evac engine per phase-A chunk (32 chunks; GPSIMD cannot read PSUM)
A_EVAC = (["A", "A", "A", "V"] * 8)


def _tap_plan(k):
    """Return list of (t, kind, rp, par, dlt).

    kind in {"P", "V", "G"}.  Border blocks read host-prepared virtual
    tiles (64/65: block-0 i<2, upper half only; 66: block-31 i=4, lower
    half only; the missing half is zeros and its weights are bmask-zeroed),
    so every tap is a full-128-partition op with delta in {-1, 0, +1}
    realized on PE as a (shifted-)diagonal lhsT.
    """
    plan = []
    for t in range(25):
        i, j = divmod(t, KK)
        par = j & 1
        dlt = DELTA[j]
        if k == 0 and i < 2:
            rp = 64 + i
        elif k == 31 and i == 4:
            rp = 66
        else:
            rp = 2 * k + i - 2
        plan.append((t, TAP_ENGINE[t], rp, par, dlt))
    return plan


def _build_nc():
    nc = bacc.Bacc(None, target_bir_lowering=False, debug=False)

    xc_d = nc.declare_dram_parameter("xc", [2, 128, H * W], DTK, isOutput=False)
    xt_d = nc.declare_dram_parameter("xt", [68, 128, 2, C], DTX, isOutput=False)
    wc_d = nc.declare_dram_parameter("wc", [2, 128, CC], DTK, isOutput=False)
    bc_d = nc.declare_dram_parameter("bc", [CC, 1], F32, isOutput=False)
    wt_d = nc.declare_dram_parameter("wt", [CC, 9, 25], DTK, isOutput=False)
    be_d = nc.declare_dram_parameter("be", [25, 1], F32, isOutput=False)
    id_d = nc.declare_dram_parameter("idn", [25, 25], DTK, isOutput=False)
    i128_d = nc.declare_dram_parameter("i128", [128, 128], DTX, isOutput=False)
    isup_d = nc.declare_dram_parameter("isup", [128, 128], DTX, isOutput=False)
    isub_d = nc.declare_dram_parameter("isub", [128, 128], DTX, isOutput=False)
    bm_d = nc.declare_dram_parameter("bm", [3, 128, 25], F32, isOutput=False)
    out_d = nc.declare_dram_parameter("out", [NB, 128, C], DTX, isOutput=True)

    CXW = 130  # padded cx row length

    with tile.TileContext(nc) as tc:
        with (
            tc.tile_pool(name="consts", bufs=1) as consts,
            tc.tile_pool(name="xtp", bufs=17) as xtp,
            tc.tile_pool(name="xcin", bufs=3) as xcin,
            tc.tile_pool(name="cx", bufs=1) as cxpool,
            tc.tile_pool(name="psA", bufs=2, space="PSUM") as psA,
            tc.tile_pool(name="psM", bufs=2, space="PSUM") as psM,
            tc.tile_pool(name="psT", bufs=1, space="PSUM") as psT,
            tc.tile_pool(name="psO", bufs=2, space="PSUM") as psO,
            tc.tile_pool(name="soft", bufs=6) as soft,
            tc.tile_pool(name="wmask", bufs=8) as wmask,
            tc.tile_pool(name="diag", bufs=24) as diagp,
            tc.tile_pool(name="accp", bufs=8) as accp,
        ):
            # ---- constants / weights ----
            wc_sb = consts.tile([128, 2, CC], DTK)
            nc.sync.dma_start(out=wc_sb, in_=wc_d[:, :, :].rearrange("c p m -> p c m"))
            wt_sb = consts.tile([CC, 9, 25], DTK)
            nc.sync.dma_start(out=wt_sb, in_=wt_d[:, :, :])
            bc_sb = consts.tile([CC, 1], F32)
            nc.sync.dma_start(out=bc_sb, in_=bc_d[:, :])
            be_sb = consts.tile([25, 1], F32)
            nc.sync.dma_start(out=be_sb, in_=be_d[:, :])
            id_sb = consts.tile([25, 25], DTK)
            nc.sync.dma_start(out=id_sb, in_=id_d[:, :])
            i128_sb = consts.tile([128, 128], DTX)
            nc.sync.dma_start(out=i128_sb, in_=i128_d[:, :])
            isup_sb = consts.tile([128, 128], DTX)
            nc.sync.dma_start(out=isup_sb, in_=isup_d[:, :])
            isub_sb = consts.tile([128, 128], DTX)
            nc.sync.dma_start(out=isub_sb, in_=isub_d[:, :])
            bm_sb = consts.tile([128, 3, 25], F32)
            nc.sync.dma_start(out=bm_sb, in_=bm_d[:, :, :].rearrange("v p m -> p v m"))

            # ---- XT row-pair tiles, 17 groups of 4 rp (stream immediately;
            # rp 64..66 are the border-block virtual tiles, 67 is padding) ----
            xt_groups = []
            for g in range(17):
                gt = xtp.tile([128, 4, 2, C], DTX, name=f"xt{g}", tag="xt")
                nc.sync.dma_start(
                    out=gt,
                    in_=xt_d[4 * g:4 * g + 4, :, :, :].rearrange(
                        "r p j c -> p r j c"))
                xt_groups.append(gt)

            def xt_slice(rp, par):
                return xt_groups[rp // 4][:, rp % 4, par, :]

            # ---- cx_pad (compressor output, 1-px zero ring, flat layout) ----
            cx_pad = cxpool.tile([CC, CXW * CXW], DTK)
            cp = cx_pad[:, :]
            zrow = consts.tile([CC, CXW], DTK)
            nc.vector.memset(zrow, 0.0)
            nc.scalar.copy(out=cp[:, 0:CXW], in_=zrow[:, :])
            nc.scalar.copy(
                out=bass.AP(tensor=cp.tensor, offset=cp.offset + CXW,
                            ap=[cp.ap[0], [CXW, 129], [1, 1]]),
                in_=zrow[:, 0:129],
            )

            tc.strict_bb_all_engine_barrier()

            # ---- phase A: compressor 1x1 conv (PE, bf16) ----
            for j in range(32):
                xt_in = xcin.tile([128, 2, 512], DTK)
                nc.sync.dma_start(
                    out=xt_in,
                    in_=xc_d[:, :, j * 512:(j + 1) * 512].rearrange("c p n -> p c n"),
                )
                pm = psA.tile([CC, 512], F32)
                nc.tensor.matmul(pm, lhsT=wc_sb[:, 0, :], rhs=xt_in[:, 0, :],
                                 start=True, stop=False)
                nc.tensor.matmul(pm, lhsT=wc_sb[:, 1, :], rhs=xt_in[:, 1, :],
                                 start=False, stop=True)
                dst = bass.AP(tensor=cp.tensor,
                              offset=cp.offset + (4 * j + 1) * CXW + 1,
                              ap=[cp.ap[0], [CXW, 4], [1, 128]])
                src = pm[:, :].rearrange("p (r n) -> p r n", n=128)
                ev = A_EVAC[j]
                if ev == "A":
                    nc.scalar.activation(out=dst, in_=src,
                                         func=mybir.ActivationFunctionType.Identity,
                                         bias=bc_sb[:, :])
                elif ev == "V":
                    nc.vector.tensor_scalar(out=dst, in0=src,
                                            scalar1=bc_sb[:, 0:1], scalar2=None,
                                            op0=mybir.AluOpType.add)
                else:
                    nc.gpsimd.tensor_scalar(out=dst, in0=src,
                                            scalar1=bc_sb[:, 0:1], scalar2=None,
                                            op0=mybir.AluOpType.add)

            # ---- phase B: encoder 3x3/s2 conv -> m_all [25, 4096] (bf16) ----
            m_all = cxpool.tile([25, HP * WP], DTK)
            for j2 in range(8):
                pmM = psM.tile([25, 512], F32)
                ti = 0
                for di in range(3):
                    for dj in range(3):
                        rhs = bass.AP(
                            tensor=cp.tensor,
                            offset=cp.offset + (16 * j2 + di) * CXW + dj,
                            ap=[cp.ap[0], [2 * CXW, 8], [2, 64]],
                        )
                        nc.tensor.matmul(pmM, lhsT=wt_sb[:, ti, :], rhs=rhs,
                                         start=(ti == 0), stop=(ti == 8))
                        ti += 1
                nc.scalar.activation(out=m_all[:, j2 * 512:(j2 + 1) * 512],
                                     in_=pmM,
                                     func=mybir.ActivationFunctionType.Identity,
                                     bias=be_sb[:, :])

            # ---- phase C: per block: transpose + exp + softmax * bmask ----
            w_blocks = []
            for k in range(NB):
                pmT = psT.tile([128, 25], DTK)
                for half in range(2):
                    hcol = (k + 32 * half) * 64
                    nc.tensor.transpose(pmT[64 * half:64 * half + 64, :],
                                        m_all[:, hcol:hcol + 64], id_sb[:, :])
                e_k = soft.tile([128, 25], F32)
                nc.scalar.activation(out=e_k, in_=pmT,
                                     func=mybir.ActivationFunctionType.Exp)
                r_k = soft.tile([128, 1], F32)
                nc.vector.reduce_sum(out=r_k, in_=e_k, axis=mybir.AxisListType.X)
                nc.vector.reciprocal(out=r_k, in_=r_k)
                # w3 col 0 = softmax*bmask weights; col 1 = w shifted up one
                # partition (for super-diag lhsT); col 2 = shifted down.
                # memset first so the boundary rows of cols 1/2 are real
                # numbers (they multiply zero rows of isup/isub).
                w3 = wmask.tile([128, 3, 25], F32, name=f"w3_{k}", tag="w3")
                nc.gpsimd.memset(w3, 0.0)
                bmv = 1 if k == 0 else (2 if k == 31 else 0)
                nc.vector.scalar_tensor_tensor(
                    out=w3[:, 0, :], in0=e_k, scalar=r_k[:, 0:1],
                    in1=bm_sb[:, bmv, :],
                    op0=mybir.AluOpType.mult, op1=mybir.AluOpType.mult)
                nc.sync.dma_start(out=w3[0:127, 1, :], in_=w3[1:128, 0, :])
                nc.sync.dma_start(out=w3[1:128, 2, :], in_=w3[0:127, 0, :])
                w_blocks.append(w3)

            # ---- phase D: reassembly, 4-engine split per block ----
            for k in range(NB):
                w3 = w_blocks[k]
                plan = _tap_plan(k)
                pe_taps = [p for p in plan if p[1] == "P"]
                dve_taps = [p for p in plan if p[1] == "V"]
                gps_taps = [p for p in plan if p[1] == "G"]

                # --- PE path: (shifted-)diag matmuls into psum ---
                # dlt=0:  lhsT = diag(w)            = i128 * w
                # dlt=-1: lhsT[kp, p] = w[p] at p=kp+1 = isup * w_up
                # dlt=+1: lhsT[kp, p] = w[p] at p=kp-1 = isub * w_dn
                po = psO.tile([128, C], F32)
                bi = 0
                for n_, (t, _, rp, par, dlt) in enumerate(pe_taps):
                    D = diagp.tile([128, 128], DTX, name=f"D_{k}_{t}", tag="diag")
                    builder = BUILDERS[bi % len(BUILDERS)]
                    bi += 1
                    if dlt == -1:
                        src_b, sc = isup_sb[:, :], w3[:, 1, t:t + 1]
                    elif dlt == 0:
                        src_b, sc = i128_sb[:, :], w3[:, 0, t:t + 1]
                    else:
                        src_b, sc = isub_sb[:, :], w3[:, 2, t:t + 1]
                    if builder == "A":
                        nc.scalar.mul(out=D, in_=src_b, mul=sc)
                    elif builder == "G":
                        # Pool has no scalar-AP ops; broadcast the weight
                        # column along free via a stride-0 AP instead
                        scb = bass.AP(tensor=sc.tensor, offset=sc.offset,
                                      ap=[sc.ap[0], [0, 128]])
                        nc.gpsimd.tensor_tensor(out=D, in0=src_b, in1=scb,
                                                op=mybir.AluOpType.mult)
                    else:
                        nc.vector.tensor_scalar(out=D, in0=src_b,
                                                scalar1=sc, scalar2=None,
                                                op0=mybir.AluOpType.mult)
                    nc.tensor.matmul(po, lhsT=D,
                                     rhs=xt_slice(rp, par),
                                     start=(n_ == 0),
                                     stop=(n_ == len(pe_taps) - 1))

                # --- DVE / GPSIMD paths: STT chains (delta-0 taps only) ---
                def stt_chain(eng, taps, acc):
                    for ix, (t, kind, rp, par, dlt) in enumerate(taps):
                        assert dlt == 0, (k, t, dlt)
                        src_ = xt_slice(rp, par)
                        sc = w3[:, 0, t:t + 1]
                        if ix == 0:
                            eng.tensor_scalar(out=acc, in0=src_,
                                              scalar1=sc, scalar2=None,
                                              op0=mybir.AluOpType.mult)
                        else:
                            eng.scalar_tensor_tensor(
                                out=acc, in0=src_, scalar=sc, in1=acc,
                                op0=mybir.AluOpType.mult,
                                op1=mybir.AluOpType.add)

                accD = accp.tile([128, C], DTX, name=f"aD_{k}", tag="accD")
                stt_chain(nc.vector, dve_taps, accD)
                assert not gps_taps

                # --- merge + out ---
                fin = accp.tile([128, C], DTX, name=f"fin_{k}", tag="fin")
                nc.vector.tensor_tensor(out=fin, in0=accD, in1=po,
                                        op=mybir.AluOpType.add)
                nc.sync.dma_start(out=out_d[k, :, :], in_=fin)

    nc.compile()
    return nc


_NC_CACHE = None
LAST_RESULTS = None


def _get_nc():
    global _NC_CACHE
    if _NC_CACHE is None:
        _NC_CACHE = _build_nc()
    return _NC_CACHE


def _host_prep(x, w_comp, b_comp, w_enc, b_enc, power_p):
    """Build per-core input maps (numpy only)."""
    pe = float(np.exp(np.float64(power_p)))

    xc_all = np.ascontiguousarray(
        x.reshape(B, 2, 128, H * W)).astype(NPK)  # [B, 2, 128, HW]

    # XT row-pair tiles [B, 68, 128, 2, C]:
    # XT[rp][64*half + w'', par, c] = x[c, rp + 64*half, 2*w'' + par]
    # x [B, C, H, W] -> [B, rp(64), half(2), w''(64), par(2), c]
    xv = x.reshape(B, C, 2, 64, 64, 2)            # [B, c, half, rp, w'', par]
    xt_all = np.zeros((B, 68, 128, 2, C), dtype=NPX)
    xt_all[:, :64] = (xv.transpose(0, 3, 2, 4, 5, 1)
                        .reshape(B, 64, 128, 2, C)).astype(NPX)
    # virtual border tiles: 64/65 = block-0 i<2 (x rows 62/63 on the upper
    # half; lower half zero), 66 = block-31 i=4 (x row 64 on the lower half)
    for vi, row in ((64, 62), (65, 63)):
        g = x[:, :, row, :].reshape(B, C, 64, 2)          # [B, c, w'', par]
        xt_all[:, vi, 64:128, :, :] = g.transpose(0, 2, 3, 1).astype(NPX)
    g = x[:, :, 64, :].reshape(B, C, 64, 2)
    xt_all[:, 66, 0:64, :, :] = g.transpose(0, 2, 3, 1).astype(NPX)

    wc = np.ascontiguousarray(
        w_comp[:, :, 0, 0].T.reshape(2, 128, CC)).astype(NPK)
    bc = b_comp.reshape(CC, 1).astype(np.float32)
    wt = np.empty((CC, 9, 25), dtype=NPK)
    for di in range(3):
        for dj in range(3):
            wt[:, 3 * di + dj, :] = (pe * w_enc[:, :, di, dj]).T.astype(NPK)
    be = (pe * b_enc).reshape(25, 1).astype(np.float32)
    idn = np.eye(25, dtype=NPK)
    i128 = np.eye(128, dtype=NPX)
    isup = np.eye(128, 128, 1, dtype=NPX)    # isup[k, p] = 1 iff p == k+1
    isub = np.eye(128, 128, -1, dtype=NPX)   # isub[k, p] = 1 iff p == k-1

    bm = np.ones((3, 128, 25), dtype=np.float32)
    for p in range(128):
        wq = p % 64
        for i in range(KK):
            if wq == 0:
                bm[:, p, 5 * i + 0] = 0.0
                bm[:, p, 5 * i + 1] = 0.0
            if wq == 63:
                bm[:, p, 5 * i + 4] = 0.0
    bm[1, :64, 0:10] = 0.0    # block 0, h'=0: i < 2
    bm[2, 64:, 20:25] = 0.0   # block 31, h'=63: i = 4

    in_maps = []
    for b in range(B):
        in_maps.append({
            "xc": np.ascontiguousarray(xc_all[b]),
            "xt": np.ascontiguousarray(xt_all[b]),
            "wc": wc, "bc": bc, "wt": wt, "be": be, "idn": idn,
            "i128": i128, "isup": isup, "isub": isub, "bm": bm,
        })
    return in_maps


def kernel(x, w_comp, b_comp, w_enc, b_enc, power_p):
    x = np.asarray(x, dtype=np.float32)
    in_maps = _host_prep(np.asarray(x), np.asarray(w_comp), np.asarray(b_comp),
                         np.asarray(w_enc), np.asarray(b_enc),
                         np.asarray(power_p))
    nc = _get_nc()
    res = run_bass_kernel_spmd(nc, in_maps, list(range(NCORES)))
    global LAST_RESULTS
    LAST_RESULTS = res
    outs = np.stack([np.asarray(res.results[i]["out"]).astype(np.float32)
                     for i in range(NCORES)])
    # [B, 32, 128, 256] -> [B, C, 64, 64]; h' = half*32 + k, p = half*64 + w'
    out = (outs.reshape(B, NB, 2, 64, C)
               .transpose(0, 4, 2, 1, 3)
               .reshape(B, C, HP, WP))
    return np.ascontiguousarray(out.astype(np.float32))
